# revision 44
# baseline (speedup 1.0000x reference)
"""AWBNet (wo R2) Trainium2 kernel — v6 (front-loaded W1 stream).
Measured 66897 ns. Kept as fallback.
"""

import sys

import numpy as np

for _p in ("/opt/trn_rl_repo",):
    if _p not in sys.path:
        sys.path.insert(0, _p)

import concourse.bacc as bacc
import concourse.mybir as mybir
import concourse.tile as tile
from concourse import bass_utils

N_CORES = 8
B, H, W, C = 16, 512, 512, 3
SPC = B // N_CORES
PX_SAMPLE = H * W
P = 128

G_S = 21
G = SPC * G_S
NP = 3 * G
XCOLS = 12544

STAGE_COLS = (512, 1024, 2048, 2048, 2048, 2048, 1792, 1024)
NSTAGE = len(STAGE_COLS)
XR_HBM_STAGES = 4

HIST = 3 * 64 * 64
HID = 256
MOUT = 27
KT = HIST // P
MT = HID // P
W1_CH = 8
NCHW1 = KT // W1_CH

F16 = mybir.dt.float16
F32 = mybir.dt.float32
MULT = mybir.AluOpType.mult

_CACHE = {}


def _colmap(mat, i, c):
    if mat == 0:
        k = i
    elif mat == 1:
        k = 3 + i
    else:
        k = (6, 8, 7)[i]
    return 3 * k + c


def _build():
    nc = bacc.Bacc(
        "TRN2", target_bir_lowering=False, debug=False, num_devices=N_CORES
    )

    xi_d = nc.dram_tensor("xi", [NP, XCOLS], F16, kind="ExternalInput")
    w1_d = nc.dram_tensor("w1pm", [P, KT, HID], F16, kind="ExternalInput")
    hp_d = nc.dram_tensor("h_packed", [P, KT, SPC], F16, kind="ExternalInput")
    b1_d = nc.dram_tensor("b1_rep", [SPC, HID], F32, kind="ExternalInput")
    w2_d = nc.dram_tensor("w2i", [MT, P, 3 * 9], F16, kind="ExternalInput")
    b2_d = nc.dram_tensor("b2i", [NP, 9], F32, kind="ExternalInput")
    e3_d = nc.dram_tensor("e3", [SPC, 3, NP], F16, kind="ExternalInput")
    mask_d = nc.dram_tensor("maskS", [NP, G], F16, kind="ExternalInput")
    y_d = nc.dram_tensor("y_bands", [NP, XCOLS], F16, kind="ExternalOutput")

    offs = [0]
    for ncols in STAGE_COLS:
        offs.append(offs[-1] + ncols)
    assert offs[-1] == XCOLS

    with tile.TileContext(nc) as tc:
        with (
            tc.tile_pool(name="mlp", bufs=1) as mlp_pool,
            tc.tile_pool(name="w1s", bufs=1) as w1_pool,
            tc.tile_pool(name="px", bufs=1) as px_pool,
            tc.tile_pool(name="sqcr", bufs=3) as sqcr_pool,
            tc.tile_pool(name="yring", bufs=4) as y_pool,
        ):
            hp_sb = mlp_pool.tile([P, KT, SPC], F16, tag="hp", name="hp")
            nc.sync.dma_start(out=hp_sb, in_=hp_d[:, :, :])

            w1_sbs = []
            for kc in range(NCHW1):
                w1_sb = w1_pool.tile(
                    [P, W1_CH, HID], F16, tag=f"w1c{kc}", name=f"w1c{kc}"
                )
                q = nc.sync if kc % 2 == 0 else nc.scalar
                q.dma_start(out=w1_sb, in_=w1_d[:, kc * W1_CH : (kc + 1) * W1_CH, :])
                w1_sbs.append(w1_sb)

            xi_sb = px_pool.tile([NP, XCOLS], F16, tag="xi", name="xi")
            for st in range(NSTAGE):
                sl = slice(offs[st], offs[st + 1])
                nc.sync.dma_start(out=xi_sb[:, sl], in_=xi_d[:, sl])

            # xr stages 2-3 from HBM on scalar; stages 0-1 become S2S on
            # gpsimd (below) chasing xi_0/xi_1, which land ~25-28us —
            # before the MLP finishes — so the cr_0/cr_1 ops the
            # scheduler hoists ahead of the m-tail on the in-order DVE
            # queue are executable instead of stalling it until ~32us.
            xr_sb = px_pool.tile([NP, XCOLS], F16, tag="xr", name="xr")
            for st in range(2, XR_HBM_STAGES):
                sl = slice(offs[st], offs[st + 1])
                nc.scalar.dma_start(out=xr_sb[0:84, sl], in_=xi_d[42:126, sl])
                nc.scalar.dma_start(out=xr_sb[84:126, sl], in_=xi_d[0:42, sl])

            b1_sb = mlp_pool.tile([SPC, HID], F32, tag="b1", name="b1")
            nc.gpsimd.dma_start(out=b1_sb, in_=b1_d[:, :])
            w2_sb = mlp_pool.tile([P, MT, 3 * 9], F16, tag="w2", name="w2")
            nc.gpsimd.dma_start(out=w2_sb, in_=w2_d.rearrange("m p n -> p m n"))
            b2_sb = mlp_pool.tile([NP, 9], F32, tag="b2", name="b2")
            nc.gpsimd.dma_start(out=b2_sb, in_=b2_d[:, :])
            e3_sb = mlp_pool.tile([SPC, 3, NP], F16, tag="e3", name="e3")
            nc.gpsimd.dma_start(out=e3_sb, in_=e3_d[:, :, :])
            mask_sb = mlp_pool.tile([NP, G], F16, tag="mask", name="mask")
            nc.gpsimd.dma_start(out=mask_sb, in_=mask_d[:, :])
            # xr 0-1 S2S chasing xi_0/xi_1 — placed AFTER the small setup
            # DMAs so their xi-gated waits cannot delay the m-tail inputs
            # on the in-order gpsimd queue.
            for st in range(2):
                sl = slice(offs[st], offs[st + 1])
                nc.gpsimd.dma_start(out=xr_sb[0:84, sl], in_=xi_sb[42:126, sl])
                nc.gpsimd.dma_start(out=xr_sb[84:126, sl], in_=xi_sb[0:42, sl])
            for st in range(XR_HBM_STAGES, NSTAGE):
                sl = slice(offs[st], offs[st + 1])
                nc.gpsimd.dma_start(out=xr_sb[0:84, sl], in_=xi_sb[42:126, sl])
                nc.gpsimd.dma_start(out=xr_sb[84:126, sl], in_=xi_sb[0:42, sl])

            with tc.tile_pool(name="mlpps", bufs=1, space="PSUM") as mlp_psum:
                feat_ps = mlp_psum.tile([SPC, HID], F32, tag="featps", name="featps")
                for kc in range(NCHW1):
                    w1_sb = w1_sbs[kc]
                    for kk in range(W1_CH):
                        k = kc * W1_CH + kk
                        nc.tensor.matmul(
                            feat_ps,
                            hp_sb[:, k, :],
                            w1_sb[:, kk, :],
                            start=(k == 0),
                            stop=(k == KT - 1),
                        )

                feat_sb = mlp_pool.tile([SPC, HID], F32, tag="featsb", name="featsb")
                nc.vector.tensor_add(feat_sb, feat_ps, b1_sb)
                feat_r = mlp_pool.tile([SPC, HID], F16, tag="featr", name="featr")
                nc.vector.tensor_scalar(
                    feat_r, feat_sb, 0.0, None, mybir.AluOpType.max
                )

                pt_ps = mlp_psum.tile([P, 2 * 3, P], F32, tag="ptps", name="ptps")
                for mt in range(MT):
                    for i in range(3):
                        nc.tensor.matmul(
                            pt_ps[:, 3 * mt + i, 0:NP],
                            feat_r[:, mt * P : (mt + 1) * P],
                            e3_sb[:, i, :],
                            start=True,
                            stop=True,
                        )
                patt = mlp_pool.tile([P, 2 * 3, P], F16, tag="patt", name="patt")
                nc.scalar.copy(patt, pt_ps)
                msP_ps = mlp_psum.tile([NP, 9], F32, tag="msps", name="msps")
                nmm = 0
                for mt in range(MT):
                    for i in range(3):
                        nc.tensor.matmul(
                            msP_ps,
                            patt[:, 3 * mt + i, 0:NP],
                            w2_sb[:, mt, 9 * i : 9 * (i + 1)],
                            start=(nmm == 0),
                            stop=(nmm == 2 * MT + 1),
                        )
                        nmm += 1
                msP9 = mlp_pool.tile([NP, 9], F32, tag="msP9", name="msP9")
                nc.vector.tensor_add(msP9, msP_ps, b2_sb)

            lhsT = mlp_pool.tile([NP, 3, NP], F16, tag="lhsT", name="lhsT")
            for mat in range(3):
                for c in range(C):
                    nc.vector.tensor_scalar(
                        lhsT[:, mat, G * c : G * (c + 1)],
                        mask_sb,
                        msP9[:, 3 * mat + c : 3 * mat + c + 1],
                        None,
                        MULT,
                    )

            with tc.tile_pool(name="pxps", bufs=2, space="PSUM") as px_psum:
                for st in range(NSTAGE):
                    col0 = offs[st]
                    ncols = STAGE_COLS[st]
                    sl = slice(col0, col0 + ncols)

                    sq_t = sqcr_pool.tile([NP, 2048], F16, tag="sq", name=f"sq{st}")
                    nc.vector.tensor_mul(
                        sq_t[:, 0:ncols], xi_sb[:, sl], xi_sb[:, sl]
                    )
                    cr_t = sqcr_pool.tile([NP, 2048], F16, tag="cr", name=f"cr{st}")
                    nc.vector.tensor_mul(
                        cr_t[:, 0:ncols], xi_sb[:, sl], xr_sb[:, sl]
                    )

                    yc_ps = px_psum.tile([NP, 2048], F32, tag="yc", name=f"yc{st}")
                    nch = (ncols + 511) // 512
                    for mat in range(3):
                        for n in range(nch):
                            c0 = n * 512
                            c1 = min(c0 + 512, ncols)
                            if mat == 0:
                                rhs = xi_sb[:, col0 + c0 : col0 + c1]
                            elif mat == 1:
                                rhs = sq_t[:, c0:c1]
                            else:
                                rhs = cr_t[:, c0:c1]
                            nc.tensor.matmul(
                                yc_ps[:, c0:c1],
                                lhsT[:, mat, :],
                                rhs,
                                start=(mat == 0),
                                stop=(mat == 2),
                            )

                    y_sb = y_pool.tile([NP, 2048], F16, tag="ysb", name=f"y{st}")
                    nc.scalar.copy(y_sb[:, 0:ncols], yc_ps[:, 0:ncols])
                    yq = nc.sync if st % 2 == 0 else nc.scalar
                    yq.dma_start(out=y_d[:, sl], in_=y_sb[:, 0:ncols])

    nc.compile()
    return nc


def _prep_inputs(x, histogram, W1, b1, W2, b2):
    x = np.asarray(x, dtype=np.float32)
    hist = np.asarray(histogram, dtype=np.float32).reshape(B, HIST)
    W1 = np.asarray(W1, dtype=np.float32)
    b1 = np.asarray(b1, dtype=np.float32)
    W2 = np.asarray(W2, dtype=np.float32)
    b2 = np.asarray(b2, dtype=np.float32)

    w1pm = np.ascontiguousarray(
        W1.reshape(KT, P, HID).transpose(1, 0, 2)
    ).astype(np.float16)
    b1rep = np.ascontiguousarray(np.broadcast_to(b1, (SPC, HID)))
    e3 = np.zeros((SPC, 3, NP), dtype=np.float16)
    for i in range(3):
        for s in range(SPC):
            e3[s, i, 42 * i + G_S * s : 42 * i + G_S * (s + 1)] = 1.0

    cm = np.empty((3, 9), dtype=np.int64)
    for i in range(3):
        for mat in range(3):
            for c in range(C):
                cm[i, 3 * mat + c] = _colmap(mat, i, c)
    w2i = np.ascontiguousarray(
        W2.reshape(MT, P, MOUT)[:, :, cm.reshape(-1)].reshape(MT, P, 3, 9)
        .reshape(MT, P, 27)
    ).astype(np.float16)
    b2i = np.empty((NP, 9), dtype=np.float32)
    for i in range(3):
        b2i[42 * i : 42 * (i + 1), :] = b2[cm[i]]

    maskS = np.zeros((NP, G), dtype=np.float16)
    for i in range(3):
        for g in range(G):
            maskS[42 * i + g, g] = 1.0

    in_maps = []
    for core in range(N_CORES):
        xI = np.zeros((NP, XCOLS), dtype=np.float16)
        for s in range(SPC):
            xs = x[core * SPC + s].reshape(PX_SAMPLE, C)
            pad = np.zeros((G_S * XCOLS, C), dtype=np.float32)
            pad[:PX_SAMPLE] = xs
            v = pad.reshape(G_S, XCOLS, C)
            for i in range(3):
                xI[42 * i + G_S * s : 42 * i + G_S * (s + 1), :] = v[:, :, i].astype(
                    np.float16
                )

        h_core = hist[core * SPC : (core + 1) * SPC]
        hp = np.ascontiguousarray(
            h_core.reshape(SPC, KT, P).transpose(2, 1, 0)
        ).astype(np.float16)
        in_maps.append(
            {
                "xi": xI,
                "w1pm": w1pm,
                "h_packed": hp,
                "b1_rep": b1rep,
                "w2i": w2i,
                "b2i": b2i,
                "e3": e3,
                "maskS": maskS,
            }
        )
    return in_maps


def _unpack_output(res):
    y = np.empty((B, H, W, C), dtype=np.float32)
    for core in range(N_CORES):
        yb = np.asarray(res.results[core]["y_bands"])
        for s in range(SPC):
            v = yb[:, :].reshape(3, G, XCOLS)[:, G_S * s : G_S * (s + 1), :]
            flat = v.transpose(1, 2, 0).reshape(G_S * XCOLS, C)[:PX_SAMPLE]
            y[core * SPC + s] = flat.reshape(H, W, C).astype(np.float32)
    return y


def run(trace=False, **inputs):
    if "nc" not in _CACHE:
        _CACHE["nc"] = _build()
    nc = _CACHE["nc"]
    in_maps = _prep_inputs(**inputs)
    res = bass_utils.run_bass_kernel_spmd(
        nc, in_maps, core_ids=list(range(N_CORES)), trace=trace
    )
    y = _unpack_output(res)
    return y, res


def kernel(**inputs) -> np.ndarray:
    y, _ = run(trace=False, **inputs)
    return y


# revision 46
# speedup vs baseline: 1.0556x; 1.0556x over previous
"""AWBNet (wo R2) Trainium2 kernel — v6 (front-loaded W1 stream).
Measured 66897 ns. Kept as fallback.
"""

import sys

import numpy as np

for _p in ("/opt/trn_rl_repo",):
    if _p not in sys.path:
        sys.path.insert(0, _p)

import concourse.bacc as bacc
import concourse.mybir as mybir
import concourse.tile as tile
from concourse import bass_utils

N_CORES = 8
B, H, W, C = 16, 512, 512, 3
SPC = B // N_CORES
PX_SAMPLE = H * W
P = 128

G_S = 21
G = SPC * G_S
NP = 3 * G
XCOLS = 12544

STAGE_COLS = (512, 1024, 2048, 2048, 2048, 2048, 1792, 1024)
NSTAGE = len(STAGE_COLS)
XR_HBM_STAGES = 4

HIST = 3 * 64 * 64
HID = 256
MOUT = 27
KT = HIST // P
MT = HID // P
W1_CH = 8
NCHW1 = KT // W1_CH

F16 = mybir.dt.float16
F32 = mybir.dt.float32
MULT = mybir.AluOpType.mult

_CACHE = {}


def _colmap(mat, i, c):
    if mat == 0:
        k = i
    elif mat == 1:
        k = 3 + i
    else:
        k = (6, 8, 7)[i]
    return 3 * k + c


def _build():
    nc = bacc.Bacc(
        "TRN2", target_bir_lowering=False, debug=False, num_devices=N_CORES
    )

    xi_d = nc.dram_tensor("xi", [NP, XCOLS], F16, kind="ExternalInput")
    w1_d = nc.dram_tensor("w1pm", [P, KT, HID], F16, kind="ExternalInput")
    hp_d = nc.dram_tensor("h_packed", [P, KT, SPC], F16, kind="ExternalInput")
    b1_d = nc.dram_tensor("b1_rep", [SPC, HID], F32, kind="ExternalInput")
    w2_d = nc.dram_tensor("w2i", [MT, P, 3 * 9], F16, kind="ExternalInput")
    b2_d = nc.dram_tensor("b2i", [NP, 9], F32, kind="ExternalInput")
    e3_d = nc.dram_tensor("e3", [SPC, 3, NP], F16, kind="ExternalInput")
    mask_d = nc.dram_tensor("maskS", [NP, G], F16, kind="ExternalInput")
    y_d = nc.dram_tensor("y_bands", [NP, XCOLS], F16, kind="ExternalOutput")

    offs = [0]
    for ncols in STAGE_COLS:
        offs.append(offs[-1] + ncols)
    assert offs[-1] == XCOLS

    with tile.TileContext(nc) as tc:
        with (
            tc.tile_pool(name="mlp", bufs=1) as mlp_pool,
            tc.tile_pool(name="w1s", bufs=1) as w1_pool,
            tc.tile_pool(name="px", bufs=1) as px_pool,
            tc.tile_pool(name="sqcr", bufs=3) as sqcr_pool,
            tc.tile_pool(name="yring", bufs=4) as y_pool,
        ):
            hp_sb = mlp_pool.tile([P, KT, SPC], F16, tag="hp", name="hp")
            nc.sync.dma_start(out=hp_sb, in_=hp_d[:, :, :])

            w1_sbs = []
            for kc in range(NCHW1):
                w1_sb = w1_pool.tile(
                    [P, W1_CH, HID], F16, tag=f"w1c{kc}", name=f"w1c{kc}"
                )
                q = nc.sync if kc % 2 == 0 else nc.scalar
                q.dma_start(out=w1_sb, in_=w1_d[:, kc * W1_CH : (kc + 1) * W1_CH, :])
                w1_sbs.append(w1_sb)

            xi_sb = px_pool.tile([NP, XCOLS], F16, tag="xi", name="xi")
            for st in range(NSTAGE):
                sl = slice(offs[st], offs[st + 1])
                nc.sync.dma_start(out=xi_sb[:, sl], in_=xi_d[:, sl])

            xr_sb = px_pool.tile([NP, XCOLS], F16, tag="xr", name="xr")
            for st in range(XR_HBM_STAGES):
                sl = slice(offs[st], offs[st + 1])
                nc.scalar.dma_start(out=xr_sb[0:84, sl], in_=xi_d[42:126, sl])
                nc.scalar.dma_start(out=xr_sb[84:126, sl], in_=xi_d[0:42, sl])

            b1_sb = mlp_pool.tile([SPC, HID], F32, tag="b1", name="b1")
            nc.gpsimd.dma_start(out=b1_sb, in_=b1_d[:, :])
            w2_sb = mlp_pool.tile([P, MT, 3 * 9], F16, tag="w2", name="w2")
            nc.gpsimd.dma_start(out=w2_sb, in_=w2_d.rearrange("m p n -> p m n"))
            b2_sb = mlp_pool.tile([NP, 9], F32, tag="b2", name="b2")
            nc.gpsimd.dma_start(out=b2_sb, in_=b2_d[:, :])
            e3_sb = mlp_pool.tile([SPC, 3, NP], F16, tag="e3", name="e3")
            nc.gpsimd.dma_start(out=e3_sb, in_=e3_d[:, :, :])
            mask_sb = mlp_pool.tile([NP, G], F16, tag="mask", name="mask")
            nc.gpsimd.dma_start(out=mask_sb, in_=mask_d[:, :])
            for st in range(XR_HBM_STAGES, NSTAGE):
                sl = slice(offs[st], offs[st + 1])
                nc.gpsimd.dma_start(out=xr_sb[0:84, sl], in_=xi_sb[42:126, sl])
                nc.gpsimd.dma_start(out=xr_sb[84:126, sl], in_=xi_sb[0:42, sl])

            with tc.tile_pool(name="mlpps", bufs=1, space="PSUM") as mlp_psum:
                feat_ps = mlp_psum.tile([SPC, HID], F32, tag="featps", name="featps")
                for kc in range(NCHW1):
                    w1_sb = w1_sbs[kc]
                    for kk in range(W1_CH):
                        k = kc * W1_CH + kk
                        nc.tensor.matmul(
                            feat_ps,
                            hp_sb[:, k, :],
                            w1_sb[:, kk, :],
                            start=(k == 0),
                            stop=(k == KT - 1),
                        )

                feat_sb = mlp_pool.tile([SPC, HID], F32, tag="featsb", name="featsb")
                nc.vector.tensor_add(feat_sb, feat_ps, b1_sb)
                feat_r = mlp_pool.tile([SPC, HID], F16, tag="featr", name="featr")
                nc.vector.tensor_scalar(
                    feat_r, feat_sb, 0.0, None, mybir.AluOpType.max
                )

                pt_ps = mlp_psum.tile([P, 2 * 3, P], F32, tag="ptps", name="ptps")
                for mt in range(MT):
                    for i in range(3):
                        nc.tensor.matmul(
                            pt_ps[:, 3 * mt + i, 0:NP],
                            feat_r[:, mt * P : (mt + 1) * P],
                            e3_sb[:, i, :],
                            start=True,
                            stop=True,
                        )
                patt = mlp_pool.tile([P, 2 * 3, P], F16, tag="patt", name="patt")
                nc.scalar.copy(patt, pt_ps)
                msP_ps = mlp_psum.tile([NP, 9], F32, tag="msps", name="msps")
                nmm = 0
                for mt in range(MT):
                    for i in range(3):
                        nc.tensor.matmul(
                            msP_ps,
                            patt[:, 3 * mt + i, 0:NP],
                            w2_sb[:, mt, 9 * i : 9 * (i + 1)],
                            start=(nmm == 0),
                            stop=(nmm == 2 * MT + 1),
                        )
                        nmm += 1
                msP9 = mlp_pool.tile([NP, 9], F32, tag="msP9", name="msP9")
                nc.vector.tensor_add(msP9, msP_ps, b2_sb)

            lhsT = mlp_pool.tile([NP, 3, NP], F16, tag="lhsT", name="lhsT")
            for mat in range(3):
                for c in range(C):
                    nc.vector.tensor_scalar(
                        lhsT[:, mat, G * c : G * (c + 1)],
                        mask_sb,
                        msP9[:, 3 * mat + c : 3 * mat + c + 1],
                        None,
                        MULT,
                    )

            with tc.tile_pool(name="pxps", bufs=2, space="PSUM") as px_psum:
                for st in range(NSTAGE):
                    col0 = offs[st]
                    ncols = STAGE_COLS[st]
                    sl = slice(col0, col0 + ncols)

                    # tile_wait_until pushes these ops' scheduler-sim
                    # readiness beyond the whole kernel span, so the
                    # complete m-tail chain is emitted BEFORE any sq/cr
                    # on the in-order DVE queue (no runtime delay — real
                    # start is still gated by the xi/xr semaphores).
                    # Without this, the scheduler's optimistic DMA model
                    # hoists sq/cr of early stages ahead of the m-tail
                    # and a late xi/xr chunk stalls it for 5-7us.
                    sq_t = sqcr_pool.tile([NP, 2048], F16, tag="sq", name=f"sq{st}")
                    cr_t = sqcr_pool.tile([NP, 2048], F16, tag="cr", name=f"cr{st}")
                    with tc.tile_wait_until(0.1 + 0.001 * st):
                        nc.vector.tensor_mul(
                            sq_t[:, 0:ncols], xi_sb[:, sl], xi_sb[:, sl]
                        )
                        nc.vector.tensor_mul(
                            cr_t[:, 0:ncols], xi_sb[:, sl], xr_sb[:, sl]
                        )

                    yc_ps = px_psum.tile([NP, 2048], F32, tag="yc", name=f"yc{st}")
                    nch = (ncols + 511) // 512
                    for mat in range(3):
                        for n in range(nch):
                            c0 = n * 512
                            c1 = min(c0 + 512, ncols)
                            if mat == 0:
                                rhs = xi_sb[:, col0 + c0 : col0 + c1]
                            elif mat == 1:
                                rhs = sq_t[:, c0:c1]
                            else:
                                rhs = cr_t[:, c0:c1]
                            nc.tensor.matmul(
                                yc_ps[:, c0:c1],
                                lhsT[:, mat, :],
                                rhs,
                                start=(mat == 0),
                                stop=(mat == 2),
                            )

                    y_sb = y_pool.tile([NP, 2048], F16, tag="ysb", name=f"y{st}")
                    nc.scalar.copy(y_sb[:, 0:ncols], yc_ps[:, 0:ncols])
                    yq = nc.sync if st % 2 == 0 else nc.scalar
                    yq.dma_start(out=y_d[:, sl], in_=y_sb[:, 0:ncols])

    nc.compile()
    return nc


def _prep_inputs(x, histogram, W1, b1, W2, b2):
    x = np.asarray(x, dtype=np.float32)
    hist = np.asarray(histogram, dtype=np.float32).reshape(B, HIST)
    W1 = np.asarray(W1, dtype=np.float32)
    b1 = np.asarray(b1, dtype=np.float32)
    W2 = np.asarray(W2, dtype=np.float32)
    b2 = np.asarray(b2, dtype=np.float32)

    w1pm = np.ascontiguousarray(
        W1.reshape(KT, P, HID).transpose(1, 0, 2)
    ).astype(np.float16)
    b1rep = np.ascontiguousarray(np.broadcast_to(b1, (SPC, HID)))
    e3 = np.zeros((SPC, 3, NP), dtype=np.float16)
    for i in range(3):
        for s in range(SPC):
            e3[s, i, 42 * i + G_S * s : 42 * i + G_S * (s + 1)] = 1.0

    cm = np.empty((3, 9), dtype=np.int64)
    for i in range(3):
        for mat in range(3):
            for c in range(C):
                cm[i, 3 * mat + c] = _colmap(mat, i, c)
    w2i = np.ascontiguousarray(
        W2.reshape(MT, P, MOUT)[:, :, cm.reshape(-1)].reshape(MT, P, 3, 9)
        .reshape(MT, P, 27)
    ).astype(np.float16)
    b2i = np.empty((NP, 9), dtype=np.float32)
    for i in range(3):
        b2i[42 * i : 42 * (i + 1), :] = b2[cm[i]]

    maskS = np.zeros((NP, G), dtype=np.float16)
    for i in range(3):
        for g in range(G):
            maskS[42 * i + g, g] = 1.0

    in_maps = []
    for core in range(N_CORES):
        xI = np.zeros((NP, XCOLS), dtype=np.float16)
        for s in range(SPC):
            xs = x[core * SPC + s].reshape(PX_SAMPLE, C)
            pad = np.zeros((G_S * XCOLS, C), dtype=np.float32)
            pad[:PX_SAMPLE] = xs
            v = pad.reshape(G_S, XCOLS, C)
            for i in range(3):
                xI[42 * i + G_S * s : 42 * i + G_S * (s + 1), :] = v[:, :, i].astype(
                    np.float16
                )

        h_core = hist[core * SPC : (core + 1) * SPC]
        hp = np.ascontiguousarray(
            h_core.reshape(SPC, KT, P).transpose(2, 1, 0)
        ).astype(np.float16)
        in_maps.append(
            {
                "xi": xI,
                "w1pm": w1pm,
                "h_packed": hp,
                "b1_rep": b1rep,
                "w2i": w2i,
                "b2i": b2i,
                "e3": e3,
                "maskS": maskS,
            }
        )
    return in_maps


def _unpack_output(res):
    y = np.empty((B, H, W, C), dtype=np.float32)
    for core in range(N_CORES):
        yb = np.asarray(res.results[core]["y_bands"])
        for s in range(SPC):
            v = yb[:, :].reshape(3, G, XCOLS)[:, G_S * s : G_S * (s + 1), :]
            flat = v.transpose(1, 2, 0).reshape(G_S * XCOLS, C)[:PX_SAMPLE]
            y[core * SPC + s] = flat.reshape(H, W, C).astype(np.float32)
    return y


def run(trace=False, **inputs):
    if "nc" not in _CACHE:
        _CACHE["nc"] = _build()
    nc = _CACHE["nc"]
    in_maps = _prep_inputs(**inputs)
    res = bass_utils.run_bass_kernel_spmd(
        nc, in_maps, core_ids=list(range(N_CORES)), trace=trace
    )
    y = _unpack_output(res)
    return y, res


def kernel(**inputs) -> np.ndarray:
    y, _ = run(trace=False, **inputs)
    return y


# revision 48
# speedup vs baseline: 1.0707x; 1.0143x over previous
"""AWBNet (wo R2) Trainium2 kernel — v19 (v6 layout + scheduler gates).

Math (per sample b):
  m = reshape(relu(hist_flat @ W1 + b1) @ W2 + b2, [9, 3])
  feats(px) = [r, g, b, r^2, g^2, b^2, rg, rb, gb]
  y[px, c] = sum_k feats[px, k] * m[k, c]

8 cores, pure data parallel over batch (2 samples/core, W1 replicated;
cross-core collectives cost 60-90us on this backend). Per core:

  * SBUF partitions hold (band i, pixel group g): 3x42 = 126 rows.
  * 6.3MB W1 front-loaded in 8-k-tile chunks alternating the two HWDGE
    rings, each ring's DRAM reads kept sequential; MLP matmuls chase
    the chunks in k-order.
  * sync ring: hp, W1 evens, xi stage slices, y evens.
    scalar ring: W1 odds, xr stages 0-3 (band-rotated xi_d rows),
    y odds.  gpsimd: small tensors, xr stages 4-7 SBUF->SBUF.
  * m-tail: +b1, relu, 6 pattern matmuls (feat^T onto 126 partitions
    via per-core selector e3), patt evict (ACT), 6 band matmuls, +b2,
    9 masked tensor_scalar builds of the three 126x126 block-diagonal
    stationaries carrying m.
  * storm: 8 tapered column stages; DVE computes sq=xi*xi, cr=xi*xr;
    3 accumulating matmuls per 512-col chunk; ACT evicts; y DMAs
    alternate rings.
  * KEY: all storm sq/cr ops sit under tc.tile_wait_until(0.1ms+), a
    scheduler-sim readiness gate (zero runtime cost) that forces the
    complete m-tail to be emitted ahead of them on the in-order DVE
    queue. Without it the scheduler's optimistic DMA model hoists
    early-stage sq/cr above the m-tail and a late xi/xr chunk stalls
    the chain 5-7us.

Measured 65409 ns (baseline 73584; prior best 66897). rel err 1.25e-3.
"""

import sys

import numpy as np

for _p in ("/opt/trn_rl_repo",):
    if _p not in sys.path:
        sys.path.insert(0, _p)

import concourse.bacc as bacc
import concourse.mybir as mybir
import concourse.tile as tile
from concourse import bass_utils

N_CORES = 8
B, H, W, C = 16, 512, 512, 3
SPC = B // N_CORES
PX_SAMPLE = H * W
P = 128

G_S = 21
G = SPC * G_S
NP = 3 * G
XCOLS = 12544

STAGE_COLS = (512, 1024, 2048, 2048, 2048, 2048, 1792, 1024)
NSTAGE = len(STAGE_COLS)
XR_HBM_STAGES = 4

HIST = 3 * 64 * 64
HID = 256
MOUT = 27
KT = HIST // P
MT = HID // P
W1_CH = 8
NCHW1 = KT // W1_CH

F16 = mybir.dt.float16
F32 = mybir.dt.float32
MULT = mybir.AluOpType.mult

_CACHE = {}


def _colmap(mat, i, c):
    if mat == 0:
        k = i
    elif mat == 1:
        k = 3 + i
    else:
        k = (6, 8, 7)[i]
    return 3 * k + c


def _build():
    nc = bacc.Bacc(
        "TRN2", target_bir_lowering=False, debug=False, num_devices=N_CORES
    )

    xi_d = nc.dram_tensor("xi", [NP, XCOLS], F16, kind="ExternalInput")
    w1_d = nc.dram_tensor("w1pm", [P, KT, HID], F16, kind="ExternalInput")
    hp_d = nc.dram_tensor("h_packed", [P, KT, SPC], F16, kind="ExternalInput")
    b1_d = nc.dram_tensor("b1_rep", [SPC, HID], F32, kind="ExternalInput")
    w2_d = nc.dram_tensor("w2i", [MT, P, 3 * 9], F16, kind="ExternalInput")
    b2_d = nc.dram_tensor("b2i", [NP, 9], F32, kind="ExternalInput")
    e3_d = nc.dram_tensor("e3", [SPC, 3, NP], F16, kind="ExternalInput")
    mask_d = nc.dram_tensor("maskS", [NP, G], F16, kind="ExternalInput")
    y_d = nc.dram_tensor("y_bands", [NP, XCOLS], F16, kind="ExternalOutput")

    offs = [0]
    for ncols in STAGE_COLS:
        offs.append(offs[-1] + ncols)
    assert offs[-1] == XCOLS

    with tile.TileContext(nc) as tc:
        with (
            tc.tile_pool(name="mlp", bufs=1) as mlp_pool,
            tc.tile_pool(name="w1s", bufs=1) as w1_pool,
            tc.tile_pool(name="px", bufs=1) as px_pool,
            tc.tile_pool(name="sqcr", bufs=3) as sqcr_pool,
            tc.tile_pool(name="yring", bufs=4) as y_pool,
        ):
            hp_sb = mlp_pool.tile([P, KT, SPC], F16, tag="hp", name="hp")
            nc.sync.dma_start(out=hp_sb, in_=hp_d[:, :, :])

            w1_sbs = []
            for kc in range(NCHW1):
                w1_sb = w1_pool.tile(
                    [P, W1_CH, HID], F16, tag=f"w1c{kc}", name=f"w1c{kc}"
                )
                q = nc.sync if kc % 2 == 0 else nc.scalar
                q.dma_start(out=w1_sb, in_=w1_d[:, kc * W1_CH : (kc + 1) * W1_CH, :])
                w1_sbs.append(w1_sb)

            xi_sb = px_pool.tile([NP, XCOLS], F16, tag="xi", name="xi")
            for st in range(NSTAGE):
                sl = slice(offs[st], offs[st + 1])
                nc.sync.dma_start(out=xi_sb[:, sl], in_=xi_d[:, sl])

            xr_sb = px_pool.tile([NP, XCOLS], F16, tag="xr", name="xr")
            for st in range(XR_HBM_STAGES):
                sl = slice(offs[st], offs[st + 1])
                nc.scalar.dma_start(out=xr_sb[0:84, sl], in_=xi_d[42:126, sl])
                nc.scalar.dma_start(out=xr_sb[84:126, sl], in_=xi_d[0:42, sl])

            b1_sb = mlp_pool.tile([SPC, HID], F32, tag="b1", name="b1")
            nc.gpsimd.dma_start(out=b1_sb, in_=b1_d[:, :])
            w2_sb = mlp_pool.tile([P, MT, 3 * 9], F16, tag="w2", name="w2")
            nc.gpsimd.dma_start(out=w2_sb, in_=w2_d.rearrange("m p n -> p m n"))
            b2_sb = mlp_pool.tile([NP, 9], F32, tag="b2", name="b2")
            nc.gpsimd.dma_start(out=b2_sb, in_=b2_d[:, :])
            e3_sb = mlp_pool.tile([SPC, 3, NP], F16, tag="e3", name="e3")
            nc.gpsimd.dma_start(out=e3_sb, in_=e3_d[:, :, :])
            mask_sb = mlp_pool.tile([NP, G], F16, tag="mask", name="mask")
            nc.gpsimd.dma_start(out=mask_sb, in_=mask_d[:, :])
            for st in range(XR_HBM_STAGES, NSTAGE):
                sl = slice(offs[st], offs[st + 1])
                nc.gpsimd.dma_start(out=xr_sb[0:84, sl], in_=xi_sb[42:126, sl])
                nc.gpsimd.dma_start(out=xr_sb[84:126, sl], in_=xi_sb[0:42, sl])

            with tc.tile_pool(name="mlpps", bufs=1, space="PSUM") as mlp_psum:
                feat_ps = mlp_psum.tile([SPC, HID], F32, tag="featps", name="featps")
                for kc in range(NCHW1):
                    w1_sb = w1_sbs[kc]
                    for kk in range(W1_CH):
                        k = kc * W1_CH + kk
                        nc.tensor.matmul(
                            feat_ps,
                            hp_sb[:, k, :],
                            w1_sb[:, kk, :],
                            start=(k == 0),
                            stop=(k == KT - 1),
                        )

                feat_sb = mlp_pool.tile([SPC, HID], F32, tag="featsb", name="featsb")
                nc.vector.tensor_add(feat_sb, feat_ps, b1_sb)
                feat_r = mlp_pool.tile([SPC, HID], F16, tag="featr", name="featr")
                nc.vector.tensor_scalar(
                    feat_r, feat_sb, 0.0, None, mybir.AluOpType.max
                )

                pt_ps = mlp_psum.tile([P, 2 * 3, P], F32, tag="ptps", name="ptps")
                for mt in range(MT):
                    for i in range(3):
                        nc.tensor.matmul(
                            pt_ps[:, 3 * mt + i, 0:NP],
                            feat_r[:, mt * P : (mt + 1) * P],
                            e3_sb[:, i, :],
                            start=True,
                            stop=True,
                        )
                # patt evict on DVE (provably idle 32-35us now that the
                # storm sq/cr are sim-gated behind the m-tail), split in
                # halves so the mt=0 band matmuls start half an evict
                # earlier. On ACT this op sat ~0.9us behind a DMA-issue
                # interloper.
                patt = mlp_pool.tile([P, 2 * 3, P], F16, tag="patt", name="patt")
                nc.vector.tensor_copy(patt[:, 0:3, :], pt_ps[:, 0:3, :])
                nc.vector.tensor_copy(patt[:, 3:6, :], pt_ps[:, 3:6, :])
                msP_ps = mlp_psum.tile([NP, 9], F32, tag="msps", name="msps")
                nmm = 0
                for mt in range(MT):
                    for i in range(3):
                        nc.tensor.matmul(
                            msP_ps,
                            patt[:, 3 * mt + i, 0:NP],
                            w2_sb[:, mt, 9 * i : 9 * (i + 1)],
                            start=(nmm == 0),
                            stop=(nmm == 2 * MT + 1),
                        )
                        nmm += 1
                msP9 = mlp_pool.tile([NP, 9], F32, tag="msP9", name="msP9")
                nc.vector.tensor_add(msP9, msP_ps, b2_sb)

            lhsT = mlp_pool.tile([NP, 3, NP], F16, tag="lhsT", name="lhsT")
            for mat in range(3):
                for c in range(C):
                    nc.vector.tensor_scalar(
                        lhsT[:, mat, G * c : G * (c + 1)],
                        mask_sb,
                        msP9[:, 3 * mat + c : 3 * mat + c + 1],
                        None,
                        MULT,
                    )

            with tc.tile_pool(name="pxps", bufs=2, space="PSUM") as px_psum:
                for st in range(NSTAGE):
                    col0 = offs[st]
                    ncols = STAGE_COLS[st]
                    sl = slice(col0, col0 + ncols)

                    # tile_wait_until pushes these ops' scheduler-sim
                    # readiness beyond the whole kernel span, so the
                    # complete m-tail chain is emitted BEFORE any sq/cr
                    # on the in-order DVE queue (no runtime delay — real
                    # start is still gated by the xi/xr semaphores).
                    # Without this, the scheduler's optimistic DMA model
                    # hoists sq/cr of early stages ahead of the m-tail
                    # and a late xi/xr chunk stalls it for 5-7us.
                    sq_t = sqcr_pool.tile([NP, 2048], F16, tag="sq", name=f"sq{st}")
                    cr_t = sqcr_pool.tile([NP, 2048], F16, tag="cr", name=f"cr{st}")
                    with tc.tile_wait_until(0.1 + 0.001 * st):
                        nc.vector.tensor_mul(
                            sq_t[:, 0:ncols], xi_sb[:, sl], xi_sb[:, sl]
                        )
                        nc.vector.tensor_mul(
                            cr_t[:, 0:ncols], xi_sb[:, sl], xr_sb[:, sl]
                        )

                    yc_ps = px_psum.tile([NP, 2048], F32, tag="yc", name=f"yc{st}")
                    nch = (ncols + 511) // 512
                    for mat in range(3):
                        for n in range(nch):
                            c0 = n * 512
                            c1 = min(c0 + 512, ncols)
                            if mat == 0:
                                rhs = xi_sb[:, col0 + c0 : col0 + c1]
                            elif mat == 1:
                                rhs = sq_t[:, c0:c1]
                            else:
                                rhs = cr_t[:, c0:c1]
                            nc.tensor.matmul(
                                yc_ps[:, c0:c1],
                                lhsT[:, mat, :],
                                rhs,
                                start=(mat == 0),
                                stop=(mat == 2),
                            )

                    y_sb = y_pool.tile([NP, 2048], F16, tag="ysb", name=f"y{st}")
                    nc.scalar.copy(y_sb[:, 0:ncols], yc_ps[:, 0:ncols])
                    yq = nc.sync if st % 2 == 0 else nc.scalar
                    yq.dma_start(out=y_d[:, sl], in_=y_sb[:, 0:ncols])

    nc.compile()
    return nc


def _prep_inputs(x, histogram, W1, b1, W2, b2):
    x = np.asarray(x, dtype=np.float32)
    hist = np.asarray(histogram, dtype=np.float32).reshape(B, HIST)
    W1 = np.asarray(W1, dtype=np.float32)
    b1 = np.asarray(b1, dtype=np.float32)
    W2 = np.asarray(W2, dtype=np.float32)
    b2 = np.asarray(b2, dtype=np.float32)

    w1pm = np.ascontiguousarray(
        W1.reshape(KT, P, HID).transpose(1, 0, 2)
    ).astype(np.float16)
    b1rep = np.ascontiguousarray(np.broadcast_to(b1, (SPC, HID)))
    e3 = np.zeros((SPC, 3, NP), dtype=np.float16)
    for i in range(3):
        for s in range(SPC):
            e3[s, i, 42 * i + G_S * s : 42 * i + G_S * (s + 1)] = 1.0

    cm = np.empty((3, 9), dtype=np.int64)
    for i in range(3):
        for mat in range(3):
            for c in range(C):
                cm[i, 3 * mat + c] = _colmap(mat, i, c)
    w2i = np.ascontiguousarray(
        W2.reshape(MT, P, MOUT)[:, :, cm.reshape(-1)].reshape(MT, P, 3, 9)
        .reshape(MT, P, 27)
    ).astype(np.float16)
    b2i = np.empty((NP, 9), dtype=np.float32)
    for i in range(3):
        b2i[42 * i : 42 * (i + 1), :] = b2[cm[i]]

    maskS = np.zeros((NP, G), dtype=np.float16)
    for i in range(3):
        for g in range(G):
            maskS[42 * i + g, g] = 1.0

    in_maps = []
    for core in range(N_CORES):
        xI = np.zeros((NP, XCOLS), dtype=np.float16)
        for s in range(SPC):
            xs = x[core * SPC + s].reshape(PX_SAMPLE, C)
            pad = np.zeros((G_S * XCOLS, C), dtype=np.float32)
            pad[:PX_SAMPLE] = xs
            v = pad.reshape(G_S, XCOLS, C)
            for i in range(3):
                xI[42 * i + G_S * s : 42 * i + G_S * (s + 1), :] = v[:, :, i].astype(
                    np.float16
                )

        h_core = hist[core * SPC : (core + 1) * SPC]
        hp = np.ascontiguousarray(
            h_core.reshape(SPC, KT, P).transpose(2, 1, 0)
        ).astype(np.float16)
        in_maps.append(
            {
                "xi": xI,
                "w1pm": w1pm,
                "h_packed": hp,
                "b1_rep": b1rep,
                "w2i": w2i,
                "b2i": b2i,
                "e3": e3,
                "maskS": maskS,
            }
        )
    return in_maps


def _unpack_output(res):
    y = np.empty((B, H, W, C), dtype=np.float32)
    for core in range(N_CORES):
        yb = np.asarray(res.results[core]["y_bands"])
        for s in range(SPC):
            v = yb[:, :].reshape(3, G, XCOLS)[:, G_S * s : G_S * (s + 1), :]
            flat = v.transpose(1, 2, 0).reshape(G_S * XCOLS, C)[:PX_SAMPLE]
            y[core * SPC + s] = flat.reshape(H, W, C).astype(np.float32)
    return y


def run(trace=False, **inputs):
    if "nc" not in _CACHE:
        _CACHE["nc"] = _build()
    nc = _CACHE["nc"]
    in_maps = _prep_inputs(**inputs)
    res = bass_utils.run_bass_kernel_spmd(
        nc, in_maps, core_ids=list(range(N_CORES)), trace=trace
    )
    y = _unpack_output(res)
    return y, res


def kernel(**inputs) -> np.ndarray:
    y, _ = run(trace=False, **inputs)
    return y
